# revision 20
# baseline (speedup 1.0000x reference)
"""ACELoss3D distributed Trainium2 kernel.

Strategy: pure data-parallel over 8 NeuronCores. The D spatial axis (size 128)
is sharded 8 x 16 with clamp-replicated +-1 halos sliced on the host (clamp
padding reproduces the reference's one-sided boundary formulas exactly, so all
cores run an identical interior-stencil kernel). Inputs are cast to fp16 on the
host (halves DMA traffic; validated ~4e-6 rel err vs the f32 reference).

Device layout per core: 6 images (2x3 batch) x 18 d-slabs; SBUF tiles are
[128 H-partitions, d-blocks x 130] where each 130-wide W block carries
clamp-replicated edge pads so every W stencil is the plain interior stencil.
H-axis derivatives (cj, cjj, cij) are PE matmuls against exact clamped stencil
matrices; D and W derivatives are 2x-mode fp16 DVE ops on +-1-block / +-2-elem
aligned slices. All global sums land in per-partition fp32 accumulators via
fused accum_out; the host reduces the 8 x [128,24] partials.
"""
import sys

sys.path.insert(0, '/opt/trn_rl_repo')

import numpy as np

N_CORES = 8
D_SH = 16          # d-slab owned per core
IMGS = 6           # 2*3 leading dims flattened
ALPHA, BETA, MIU, EPS = 0.001, 1.0, 1.0, 1e-8

_CACHE = {}


def _stencils():
    I = np.eye(128, dtype=np.float64)
    up = I[np.minimum(np.arange(128) + 1, 127)]
    dn = I[np.maximum(np.arange(128) - 1, 0)]
    d1 = (0.5 * (up - dn)).astype(np.float16)
    d2 = (up + dn - 2 * I).astype(np.float16)
    # matmul computes out = lhsT.T @ rhs, so pass D.T as lhsT
    return np.ascontiguousarray(d1.T), np.ascontiguousarray(d2.T)


def _build():
    import concourse.mybir as mybir
    from concourse import bacc
    from concourse.tile import TileContext
    from contextlib import ExitStack

    F16, F32 = mybir.dt.float16, mybir.dt.float32
    Alu = mybir.AluOpType
    Act = mybir.ActivationFunctionType

    nc = bacc.Bacc("TRN2", target_bir_lowering=False, debug=False,
                   num_devices=N_CORES)
    yp = nc.dram_tensor("yp", [IMGS, D_SH + 2, 128, 128], F16,
                        kind="ExternalInput")
    yt = nc.dram_tensor("yt", [IMGS, D_SH, 128, 128], F16,
                        kind="ExternalInput")
    d1 = nc.dram_tensor("d1", [128, 128], F16, kind="ExternalInput")
    d2 = nc.dram_tensor("d2", [128, 128], F16, kind="ExternalInput")
    out = nc.dram_tensor("out", [128, 24], F32, kind="ExternalOutput")

    ypa, yta = yp.ap(), yt.ap()

    with TileContext(nc) as tc, ExitStack() as ctx:
        cpool = ctx.enter_context(tc.tile_pool(name="const", bufs=1))
        io = ctx.enter_context(tc.tile_pool(name="io", bufs=2))
        fp = ctx.enter_context(tc.tile_pool(name="fields", bufs=1))
        sp = ctx.enter_context(tc.tile_pool(name="scr", bufs=2))
        pp = ctx.enter_context(tc.tile_pool(name="ps", bufs=2, space="PSUM"))

        d1s = cpool.tile([128, 128], F16)
        nc.sync.dma_start(d1s[:], d1.ap())
        d2s = cpool.tile([128, 128], F16)
        nc.sync.dma_start(d2s[:], d2.ap())
        acc = cpool.tile([128, 24], F32)
        ones = cpool.tile([128, 1], F16)
        nc.vector.memset(ones[:], 1.0)
        rin_ps = pp.tile([128, 512], F32, name="rin_ps", bufs=1)
        xt2_ps = pp.tile([128, 512], F32, name="xt2_ps", bufs=1)
        el_ps = pp.tile([128, 512], F32, name="el_ps", bufs=1)

        for i in range(IMGS):
            # ---- loads ----
            Xp = io.tile([128, 18, 130], F16)
            nc.sync.dma_start(Xp[:, :, 1:129], ypa[i].rearrange("d h w -> h d w"))
            nc.vector.tensor_copy(Xp[:, :, 0], Xp[:, :, 1])
            nc.vector.tensor_copy(Xp[:, :, 129], Xp[:, :, 128])
            xu = io.tile([128, 16, 128], F16)
            nc.sync.dma_start(xu[:], ypa[i, 1:17].rearrange("d h w -> h d w"))
            Tt = io.tile([128, 16, 128], F16)
            nc.sync.dma_start(Tt[:], yta[i].rearrange("d h w -> h d w"))

            # ---- D-axis derivatives (forward-diff route, all 2x aligned) ----
            g = fp.tile([128, 17, 130], F16)
            nc.vector.tensor_sub(g[:], Xp[:, 1:18, :], Xp[:, 0:17, :])
            ci_raw = fp.tile([128, 16, 130], F16)
            nc.vector.tensor_add(ci_raw[:], g[:, 1:17, :], g[:, 0:16, :])
            cii = fp.tile([128, 16, 130], F16)
            nc.vector.tensor_sub(cii[:], g[:, 1:17, :], g[:, 0:16, :])

            # ---- W-axis derivatives (+-2 element offsets stay 4B-aligned) ----
            ck_raw = fp.tile([128, 16, 128], F16)
            nc.vector.tensor_sub(ck_raw[:], Xp[:, 1:17, 2:130], Xp[:, 1:17, 0:128])
            Aw = fp.tile([128, 16, 128], F16)
            nc.vector.tensor_add(Aw[:], Xp[:, 1:17, 2:130], Xp[:, 1:17, 0:128])
            xm2 = sp.tile([128, 16, 128], F16, name="xm2", bufs=1)
            nc.scalar.mul(xm2[:], xu[:], -2.0)
            ckk = fp.tile([128, 16, 128], F16)
            nc.vector.tensor_add(ckk[:], xm2[:], Aw[:])
            cik_raw = fp.tile([128, 16, 128], F16)
            nc.vector.tensor_sub(cik_raw[:], ci_raw[:, :, 2:130],
                                 ci_raw[:, :, 0:128])

            # ---- H-axis derivatives on the PE ----
            cj = fp.tile([128, 16, 130], F16)
            for j in range(6):          # 5 chunks of 3 blocks + 1 of 1 (390/130)
                nb = 3 if j < 5 else 1
                ps = pp.tile([128, 3, 130], F32, name="ps_cj", bufs=1)
                nc.tensor.matmul(ps[:, 0:nb, :], d1s[:],
                                 Xp[:, 1 + 3 * j:1 + 3 * j + nb, :],
                                 start=True, stop=True)
                nc.scalar.copy(cj[:, 3 * j:3 * j + nb, :], ps[:, 0:nb, :])
            cjj = fp.tile([128, 16, 128], F16)
            for j in range(4):          # 4 chunks of 4 blocks (512)
                ps2 = pp.tile([128, 4, 128], F32, name="ps_cjj")
                nc.tensor.matmul(ps2[:], d2s[:], xu[:, 4 * j:4 * j + 4, :],
                                 start=True, stop=True)
                nc.scalar.copy(cjj[:, 4 * j:4 * j + 4, :], ps2[:])
            cij = fp.tile([128, 16, 128], F16)
            for j in range(4):
                ps3 = pp.tile([128, 4, 128], F32, name="ps_cij")
                nc.tensor.matmul(ps3[:], d1s[:],
                                 ci_raw[:, 4 * j:4 * j + 4, 1:129],
                                 start=True, stop=True)
                nc.scalar.copy(cij[:, 4 * j:4 * j + 4, :], ps3[:])

            cjk = fp.tile([128, 16, 128], F16)
            nc.vector.tensor_sub(cjk[:], cj[:, :, 2:130], cj[:, :, 0:128])

            # ---- squares: a=(0.5 ci_raw)^2, b=cj^2, c=0.25 ck_raw^2 ----
            a_t = fp.tile([128, 16, 128], F16)
            nc.scalar.activation(a_t[:], ci_raw[:, :, 1:129], Act.Square,
                                 scale=0.5)
            b_t = fp.tile([128, 16, 128], F16)
            nc.scalar.activation(b_t[:], cj[:, :, 1:129], Act.Square)
            c_t = fp.tile([128, 16, 128], F16)
            nc.scalar.activation(c_t[:], ck_raw[:], Act.Square, scale=0.5)

            ab = fp.tile([128, 16, 128], F16)
            nc.vector.tensor_add(ab[:], a_t[:], b_t[:])
            bc = fp.tile([128, 16, 128], F16)
            nc.vector.tensor_add(bc[:], b_t[:], c_t[:])
            ac = fp.tile([128, 16, 128], F16)
            nc.vector.tensor_add(ac[:], a_t[:], c_t[:])
            s2 = fp.tile([128, 16, 128], F16)
            nc.vector.tensor_add(s2[:], ab[:], c_t[:])

            # curv = (1+a+b)ckk + (1+b+c)cii + (1+a+c)cjj - cik_raw*cjk*cij
            ap1 = sp.tile([128, 16, 128], F16, name="ap1", bufs=1)
            nc.scalar.add(ap1[:], ab[:], 1.0)
            A_t = fp.tile([128, 16, 128], F16)
            nc.vector.tensor_mul(A_t[:], ap1[:], ckk[:])
            B_t = fp.tile([128, 16, 128], F16)
            nc.vector.scalar_tensor_tensor(B_t[:], bc[:], 1.0, cii[:, :, 1:129],
                                           Alu.add, Alu.mult)
            cp1 = sp.tile([128, 16, 128], F16, name="cp1", bufs=1)
            nc.scalar.add(cp1[:], ac[:], 1.0)
            C_t = fp.tile([128, 16, 128], F16)
            nc.vector.tensor_mul(C_t[:], cp1[:], cjj[:])
            S_t = fp.tile([128, 16, 128], F16)
            nc.vector.tensor_add(S_t[:], A_t[:], B_t[:])
            S2_t = fp.tile([128, 16, 128], F16)
            nc.vector.tensor_add(S2_t[:], S_t[:], C_t[:])
            p1 = fp.tile([128, 16, 128], F16)
            nc.vector.tensor_mul(p1[:], cik_raw[:], cjk[:])
            p2 = fp.tile([128, 16, 128], F16)
            nc.vector.tensor_mul(p2[:], p1[:], cij[:])
            curv = fp.tile([128, 16, 128], F16)
            nc.vector.tensor_sub(curv[:], S2_t[:], p2[:])

            # contrib = curv^2 * length / (1+s)
            rec_in = sp.tile([128, 16, 128], F32, name="rec_in", bufs=1)
            nc.scalar.add(rec_in[:], s2[:], 1.0)
            r_t = sp.tile([128, 16, 128], F32, name="r_t", bufs=1)
            nc.vector.reciprocal_approx_fast(r_t[:], rec_in[:])
            ln_t = fp.tile([128, 16, 128], F16)
            nc.scalar.activation(ln_t[:], s2[:], Act.Sqrt)
            curv2 = fp.tile([128, 16, 128], F16)
            nc.scalar.activation(curv2[:], curv[:], Act.Square)
            c2l = fp.tile([128, 16, 128], F16)
            nc.vector.tensor_mul(c2l[:], curv2[:], ln_t[:])
            r16 = sp.tile([128, 16, 128], F16, name="r16", bufs=1)
            nc.scalar.copy(r16[:], r_t[:])
            contrib = fp.tile([128, 16, 128], F16)
            nc.vector.tensor_mul(contrib[:], c2l[:], r16[:])
            for j in range(4):
                nc.tensor.matmul(el_ps[0:1, :], ones[:],
                                 contrib[:, 4 * j:4 * j + 4, :],
                                 start=(i == 0 and j == 0),
                                 stop=(i == IMGS - 1 and j == 3))

            # ---- region sums: products on DVE, reduction on PE (ones-matmul
            # accumulating into a persistent PSUM row across all images) ----
            s1 = fp.tile([128, 16, 128], F16)
            nc.scalar.activation(s1[:], Tt[:], Act.Square, bias=1.0, scale=-1.0)
            rin_p = fp.tile([128, 16, 128], F16)
            nc.vector.tensor_mul(rin_p[:], xu[:], s1[:])
            s2T = fp.tile([128, 16, 128], F16)
            nc.scalar.activation(s2T[:], Tt[:], Act.Square,
                                 accum_out=acc[:, 18 + i:19 + i])
            xt2_p = fp.tile([128, 16, 128], F16)
            nc.vector.tensor_mul(xt2_p[:], xu[:], s2T[:])
            for j in range(4):
                st = (i == 0 and j == 0)
                sto = (i == IMGS - 1 and j == 3)
                nc.tensor.matmul(rin_ps[0:1, :], ones[:],
                                 rin_p[:, 4 * j:4 * j + 4, :],
                                 start=st, stop=sto)
                nc.tensor.matmul(xt2_ps[0:1, :], ones[:],
                                 xt2_p[:, 4 * j:4 * j + 4, :],
                                 start=st, stop=sto)

        nc.vector.tensor_reduce(acc[0:1, 0:1], el_ps[0:1, :],
                                mybir.AxisListType.X, Alu.add)
        nc.vector.tensor_reduce(acc[0:1, 6:7], rin_ps[0:1, :],
                                mybir.AxisListType.X, Alu.add)
        nc.vector.tensor_reduce(acc[0:1, 12:13], xt2_ps[0:1, :],
                                mybir.AxisListType.X, Alu.add)
        nc.sync.dma_start(out.ap(), acc[:])

    nc.compile()
    return nc


def _get_nc():
    if "nc" not in _CACHE:
        _CACHE["nc"] = _build()
    return _CACHE["nc"]


def kernel(y_pred: np.ndarray, y_true: np.ndarray) -> np.ndarray:
    from concourse.bass_utils import run_bass_kernel_spmd

    yp = np.asarray(y_pred).reshape(IMGS, 128, 128, 128)
    yt = np.asarray(y_true).reshape(IMGS, 128, 128, 128)
    d1t, d2t = _stencils()

    in_maps = []
    for c in range(N_CORES):
        idx = np.clip(np.arange(D_SH * c - 1, D_SH * c + D_SH + 1), 0, 127)
        in_maps.append({
            "yp": np.ascontiguousarray(yp[:, idx].astype(np.float16)),
            "yt": np.ascontiguousarray(
                yt[:, D_SH * c:D_SH * (c + 1)].astype(np.float16)),
            "d1": d1t,
            "d2": d2t,
        })

    nc = _get_nc()
    res = run_bass_kernel_spmd(nc, in_maps, core_ids=list(range(N_CORES)))

    elast = rin = t2 = xt2 = 0.0
    for c in range(N_CORES):
        o = res.results[c]["out"].astype(np.float64)
        elast += o[0, 0]
        rin += o[0, 6]
        xt2 += o[0, 12]
        t2 += o[:, 18:24].sum()

    total = (MIU * abs(rin) + abs(t2 - xt2)
             + ALPHA * yp.size + BETA * elast)
    return np.array(total, dtype=np.float32)


# revision 21
# speedup vs baseline: 1.1131x; 1.1131x over previous
"""ACELoss3D distributed Trainium2 kernel.

Strategy: pure data-parallel over 8 NeuronCores. The D spatial axis (size 128)
is sharded 8 x 16 with clamp-replicated +-1 halos sliced on the host (clamp
padding reproduces the reference's one-sided boundary formulas exactly, so all
cores run an identical interior-stencil kernel). Inputs are cast to fp16 on the
host (halves DMA traffic; validated ~4e-6 rel err vs the f32 reference).

Device layout per core: 6 images (2x3 batch) x 18 d-slabs; SBUF tiles are
[128 H-partitions, d-blocks x 130] where each 130-wide W block carries
clamp-replicated edge pads so every W stencil is the plain interior stencil.
H-axis derivatives (cj, cjj, cij) are PE matmuls against exact clamped stencil
matrices; D and W derivatives are 2x-mode fp16 DVE ops on +-1-block / +-2-elem
aligned slices. All global sums land in per-partition fp32 accumulators via
fused accum_out; the host reduces the 8 x [128,24] partials.
"""
import sys

sys.path.insert(0, '/opt/trn_rl_repo')

import numpy as np

N_CORES = 8
D_SH = 16          # d-slab owned per core
IMGS = 6           # 2*3 leading dims flattened
ALPHA, BETA, MIU, EPS = 0.001, 1.0, 1.0, 1e-8

_CACHE = {}


def _stencils():
    I = np.eye(128, dtype=np.float64)
    up = I[np.minimum(np.arange(128) + 1, 127)]
    dn = I[np.maximum(np.arange(128) - 1, 0)]
    d1 = (0.5 * (up - dn)).astype(np.float16)
    d2 = (up + dn - 2 * I).astype(np.float16)
    # matmul computes out = lhsT.T @ rhs, so pass D.T as lhsT
    return np.ascontiguousarray(d1.T), np.ascontiguousarray(d2.T)


def _build():
    import concourse.mybir as mybir
    from concourse import bacc
    from concourse.tile import TileContext
    from contextlib import ExitStack

    F16, F32 = mybir.dt.float16, mybir.dt.float32
    Alu = mybir.AluOpType
    Act = mybir.ActivationFunctionType

    nc = bacc.Bacc("TRN2", target_bir_lowering=False, debug=False,
                   num_devices=N_CORES)
    yp = nc.dram_tensor("yp", [IMGS, D_SH + 2, 128, 128], F16,
                        kind="ExternalInput")
    yt = nc.dram_tensor("yt", [IMGS, D_SH, 128, 128], F16,
                        kind="ExternalInput")
    d1 = nc.dram_tensor("d1", [128, 128], F16, kind="ExternalInput")
    d2 = nc.dram_tensor("d2", [128, 128], F16, kind="ExternalInput")
    out = nc.dram_tensor("out", [128, 24], F32, kind="ExternalOutput")

    ypa, yta = yp.ap(), yt.ap()

    with TileContext(nc) as tc, ExitStack() as ctx:
        cpool = ctx.enter_context(tc.tile_pool(name="const", bufs=1))
        io = ctx.enter_context(tc.tile_pool(name="io", bufs=2))
        fp = ctx.enter_context(tc.tile_pool(name="fields", bufs=1))
        sp = ctx.enter_context(tc.tile_pool(name="scr", bufs=2))
        pp = ctx.enter_context(tc.tile_pool(name="ps", bufs=2, space="PSUM"))

        d1s = cpool.tile([128, 128], F16)
        nc.sync.dma_start(d1s[:], d1.ap())
        d2s = cpool.tile([128, 128], F16)
        nc.sync.dma_start(d2s[:], d2.ap())
        acc = cpool.tile([128, 24], F32)
        ones = cpool.tile([128, 1], F16)
        nc.vector.memset(ones[:], 1.0)
        rin_ps = pp.tile([128, 512], F32, name="rin_ps", bufs=1)
        xt2_ps = pp.tile([128, 512], F32, name="xt2_ps", bufs=1)

        for i in range(IMGS):
            # ---- loads ----
            Xp = io.tile([128, 18, 130], F16)
            nc.sync.dma_start(Xp[:, :, 1:129], ypa[i].rearrange("d h w -> h d w"))
            nc.vector.tensor_copy(Xp[:, :, 0], Xp[:, :, 1])
            nc.vector.tensor_copy(Xp[:, :, 129], Xp[:, :, 128])
            xu = io.tile([128, 16, 128], F16)
            nc.sync.dma_start(xu[:], ypa[i, 1:17].rearrange("d h w -> h d w"))
            Tt = io.tile([128, 16, 128], F16)
            nc.sync.dma_start(Tt[:], yta[i].rearrange("d h w -> h d w"))

            # ---- D-axis derivatives (forward-diff route, all 2x aligned) ----
            g = fp.tile([128, 17, 130], F16)
            nc.vector.tensor_sub(g[:], Xp[:, 1:18, :], Xp[:, 0:17, :])
            ci_raw = fp.tile([128, 16, 130], F16)
            nc.vector.tensor_add(ci_raw[:], g[:, 1:17, :], g[:, 0:16, :])
            cii = fp.tile([128, 16, 130], F16)
            nc.vector.tensor_sub(cii[:], g[:, 1:17, :], g[:, 0:16, :])

            # ---- W-axis derivatives (+-2 element offsets stay 4B-aligned) ----
            ck_raw = fp.tile([128, 16, 128], F16)
            nc.vector.tensor_sub(ck_raw[:], Xp[:, 1:17, 2:130], Xp[:, 1:17, 0:128])
            Aw = fp.tile([128, 16, 128], F16)
            nc.vector.tensor_add(Aw[:], Xp[:, 1:17, 2:130], Xp[:, 1:17, 0:128])
            xm2 = sp.tile([128, 16, 128], F16, name="xm2", bufs=1)
            nc.vector.tensor_scalar_mul(xm2[:], xu[:], -2.0)
            ckk = fp.tile([128, 16, 128], F16)
            nc.vector.tensor_add(ckk[:], xm2[:], Aw[:])
            cik_raw = fp.tile([128, 16, 128], F16)
            nc.vector.tensor_sub(cik_raw[:], ci_raw[:, :, 2:130],
                                 ci_raw[:, :, 0:128])

            # ---- H-axis derivatives on the PE ----
            cj = fp.tile([128, 16, 130], F16)
            for j in range(6):          # 5 chunks of 3 blocks + 1 of 1 (390/130)
                nb = 3 if j < 5 else 1
                ps = pp.tile([128, 3, 130], F32, name="ps_cj", bufs=1)
                nc.tensor.matmul(ps[:, 0:nb, :], d1s[:],
                                 Xp[:, 1 + 3 * j:1 + 3 * j + nb, :],
                                 start=True, stop=True)
                nc.scalar.copy(cj[:, 3 * j:3 * j + nb, :], ps[:, 0:nb, :])
            cjj = fp.tile([128, 16, 128], F16)
            for j in range(4):          # 4 chunks of 4 blocks (512)
                ps2 = pp.tile([128, 4, 128], F32, name="ps_cjj")
                nc.tensor.matmul(ps2[:], d2s[:], xu[:, 4 * j:4 * j + 4, :],
                                 start=True, stop=True)
                nc.scalar.copy(cjj[:, 4 * j:4 * j + 4, :], ps2[:])
            cij = fp.tile([128, 16, 128], F16)
            for j in range(4):
                ps3 = pp.tile([128, 4, 128], F32, name="ps_cij")
                nc.tensor.matmul(ps3[:], d1s[:],
                                 ci_raw[:, 4 * j:4 * j + 4, 1:129],
                                 start=True, stop=True)
                nc.scalar.copy(cij[:, 4 * j:4 * j + 4, :], ps3[:])

            cjk = fp.tile([128, 16, 128], F16)
            nc.vector.tensor_sub(cjk[:], cj[:, :, 2:130], cj[:, :, 0:128])

            # ---- squares: a=(0.5 ci_raw)^2, b=cj^2, c=0.25 ck_raw^2 ----
            a_t = fp.tile([128, 16, 128], F16)
            nc.scalar.activation(a_t[:], ci_raw[:, :, 1:129], Act.Square,
                                 scale=0.5)
            b_t = fp.tile([128, 16, 128], F16)
            nc.scalar.activation(b_t[:], cj[:, :, 1:129], Act.Square)
            c_t = fp.tile([128, 16, 128], F16)
            nc.scalar.activation(c_t[:], ck_raw[:], Act.Square, scale=0.5)

            ab = fp.tile([128, 16, 128], F16)
            nc.vector.tensor_add(ab[:], a_t[:], b_t[:])
            bc = fp.tile([128, 16, 128], F16)
            nc.vector.tensor_add(bc[:], b_t[:], c_t[:])
            ac = fp.tile([128, 16, 128], F16)
            nc.vector.tensor_add(ac[:], a_t[:], c_t[:])
            s2 = fp.tile([128, 16, 128], F16)
            nc.vector.tensor_add(s2[:], ab[:], c_t[:])

            # curv = (1+a+b)ckk + (1+b+c)cii + (1+a+c)cjj - cik_raw*cjk*cij
            ap1 = sp.tile([128, 16, 128], F16, name="ap1", bufs=1)
            nc.vector.tensor_scalar_add(ap1[:], ab[:], 1.0)
            A_t = fp.tile([128, 16, 128], F16)
            nc.vector.tensor_mul(A_t[:], ap1[:], ckk[:])
            B_t = fp.tile([128, 16, 128], F16)
            nc.vector.scalar_tensor_tensor(B_t[:], bc[:], 1.0, cii[:, :, 1:129],
                                           Alu.add, Alu.mult)
            cp1 = sp.tile([128, 16, 128], F16, name="cp1", bufs=1)
            nc.vector.tensor_scalar_add(cp1[:], ac[:], 1.0)
            C_t = fp.tile([128, 16, 128], F16)
            nc.vector.tensor_mul(C_t[:], cp1[:], cjj[:])
            S_t = fp.tile([128, 16, 128], F16)
            nc.vector.tensor_add(S_t[:], A_t[:], B_t[:])
            S2_t = fp.tile([128, 16, 128], F16)
            nc.vector.tensor_add(S2_t[:], S_t[:], C_t[:])
            p1 = fp.tile([128, 16, 128], F16)
            nc.vector.tensor_mul(p1[:], cik_raw[:], cjk[:])
            p2 = fp.tile([128, 16, 128], F16)
            nc.vector.tensor_mul(p2[:], p1[:], cij[:])
            curv = fp.tile([128, 16, 128], F16)
            nc.vector.tensor_sub(curv[:], S2_t[:], p2[:])

            # contrib = curv^2 * length / (1+s)
            rec_in = sp.tile([128, 16, 128], F32, name="rec_in", bufs=1)
            nc.vector.tensor_scalar_add(rec_in[:], s2[:], 1.0)
            r_t = sp.tile([128, 16, 128], F32, name="r_t", bufs=1)
            nc.vector.reciprocal_approx_fast(r_t[:], rec_in[:])
            ln_t = fp.tile([128, 16, 128], F16)
            nc.scalar.activation(ln_t[:], s2[:], Act.Sqrt)
            curv2 = fp.tile([128, 16, 128], F16)
            nc.scalar.activation(curv2[:], curv[:], Act.Square)
            c2l = fp.tile([128, 16, 128], F16)
            nc.vector.tensor_mul(c2l[:], curv2[:], ln_t[:])
            scr = sp.tile([128, 16, 128], F16, name="scr")
            nc.vector.scalar_tensor_tensor(scr[:], c2l[:], 1.0, r_t[:],
                                           Alu.mult, Alu.mult,
                                           accum_out=acc[:, i:i + 1])

            # ---- region sums: products on DVE, reduction on PE (ones-matmul
            # accumulating into a persistent PSUM row across all images) ----
            s1 = fp.tile([128, 16, 128], F16)
            nc.scalar.activation(s1[:], Tt[:], Act.Square, bias=1.0, scale=-1.0)
            rin_p = fp.tile([128, 16, 128], F16)
            nc.vector.tensor_mul(rin_p[:], xu[:], s1[:])
            s2T = fp.tile([128, 16, 128], F16)
            nc.scalar.activation(s2T[:], Tt[:], Act.Square,
                                 accum_out=acc[:, 18 + i:19 + i])
            xt2_p = fp.tile([128, 16, 128], F16)
            nc.vector.tensor_mul(xt2_p[:], xu[:], s2T[:])
            for j in range(4):
                st = (i == 0 and j == 0)
                sto = (i == IMGS - 1 and j == 3)
                nc.tensor.matmul(rin_ps[0:1, :], ones[:],
                                 rin_p[:, 4 * j:4 * j + 4, :],
                                 start=st, stop=sto)
                nc.tensor.matmul(xt2_ps[0:1, :], ones[:],
                                 xt2_p[:, 4 * j:4 * j + 4, :],
                                 start=st, stop=sto)

        nc.vector.tensor_reduce(acc[0:1, 6:7], rin_ps[0:1, :],
                                mybir.AxisListType.X, Alu.add)
        nc.vector.tensor_reduce(acc[0:1, 12:13], xt2_ps[0:1, :],
                                mybir.AxisListType.X, Alu.add)
        nc.sync.dma_start(out.ap(), acc[:])

    nc.compile()
    return nc


def _get_nc():
    if "nc" not in _CACHE:
        _CACHE["nc"] = _build()
    return _CACHE["nc"]


def kernel(y_pred: np.ndarray, y_true: np.ndarray) -> np.ndarray:
    from concourse.bass_utils import run_bass_kernel_spmd

    yp = np.asarray(y_pred).reshape(IMGS, 128, 128, 128)
    yt = np.asarray(y_true).reshape(IMGS, 128, 128, 128)
    d1t, d2t = _stencils()

    in_maps = []
    for c in range(N_CORES):
        idx = np.clip(np.arange(D_SH * c - 1, D_SH * c + D_SH + 1), 0, 127)
        in_maps.append({
            "yp": np.ascontiguousarray(yp[:, idx].astype(np.float16)),
            "yt": np.ascontiguousarray(
                yt[:, D_SH * c:D_SH * (c + 1)].astype(np.float16)),
            "d1": d1t,
            "d2": d2t,
        })

    nc = _get_nc()
    res = run_bass_kernel_spmd(nc, in_maps, core_ids=list(range(N_CORES)))

    elast = rin = t2 = xt2 = 0.0
    for c in range(N_CORES):
        o = res.results[c]["out"].astype(np.float64)
        elast += o[:, 0:6].sum()
        rin += o[0, 6]
        xt2 += o[0, 12]
        t2 += o[:, 18:24].sum()

    total = (MIU * abs(rin) + abs(t2 - xt2)
             + ALPHA * yp.size + BETA * elast)
    return np.array(total, dtype=np.float32)


# revision 22
# speedup vs baseline: 1.1644x; 1.0461x over previous
"""ACELoss3D distributed Trainium2 kernel.

Strategy: pure data-parallel over 8 NeuronCores. The D spatial axis (size 128)
is sharded 8 x 16 with clamp-replicated +-1 halos sliced on the host (clamp
padding reproduces the reference's one-sided boundary formulas exactly, so all
cores run an identical interior-stencil kernel). Inputs are cast to fp16 on the
host (halves DMA traffic; validated ~4e-6 rel err vs the f32 reference).

Device layout per core: 6 images (2x3 batch) x 18 d-slabs; SBUF tiles are
[128 H-partitions, d-blocks x 130] where each 130-wide W block carries
clamp-replicated edge pads so every W stencil is the plain interior stencil.
H-axis derivatives (cj, cjj, cij) are PE matmuls against exact clamped stencil
matrices; D and W derivatives are 2x-mode fp16 DVE ops on +-1-block / +-2-elem
aligned slices. All global sums land in per-partition fp32 accumulators via
fused accum_out; the host reduces the 8 x [128,24] partials.
"""
import sys

sys.path.insert(0, '/opt/trn_rl_repo')

import numpy as np

N_CORES = 8
D_SH = 16          # d-slab owned per core
IMGS = 6           # 2*3 leading dims flattened
ALPHA, BETA, MIU, EPS = 0.001, 1.0, 1.0, 1e-8

_CACHE = {}


def _stencils():
    I = np.eye(128, dtype=np.float64)
    up = I[np.minimum(np.arange(128) + 1, 127)]
    dn = I[np.maximum(np.arange(128) - 1, 0)]
    d1 = (0.5 * (up - dn)).astype(np.float16)
    d2 = (up + dn - 2 * I).astype(np.float16)
    # matmul computes out = lhsT.T @ rhs, so pass D.T as lhsT
    return np.ascontiguousarray(d1.T), np.ascontiguousarray(d2.T)


def _build():
    import concourse.mybir as mybir
    from concourse import bacc
    from concourse.tile import TileContext
    from contextlib import ExitStack

    F16, F32 = mybir.dt.float16, mybir.dt.float32
    Alu = mybir.AluOpType
    Act = mybir.ActivationFunctionType

    nc = bacc.Bacc("TRN2", target_bir_lowering=False, debug=False,
                   num_devices=N_CORES)
    yp = nc.dram_tensor("yp", [IMGS, D_SH + 2, 128, 128], F16,
                        kind="ExternalInput")
    yt = nc.dram_tensor("yt", [IMGS, D_SH, 128, 128], F16,
                        kind="ExternalInput")
    d1 = nc.dram_tensor("d1", [128, 128], F16, kind="ExternalInput")
    d2 = nc.dram_tensor("d2", [128, 128], F16, kind="ExternalInput")
    out = nc.dram_tensor("out", [128, 24], F32, kind="ExternalOutput")

    ypa, yta = yp.ap(), yt.ap()

    with TileContext(nc) as tc, ExitStack() as ctx:
        cpool = ctx.enter_context(tc.tile_pool(name="const", bufs=1))
        io = ctx.enter_context(tc.tile_pool(name="io", bufs=2))
        fp = ctx.enter_context(tc.tile_pool(name="fields", bufs=1))
        sp = ctx.enter_context(tc.tile_pool(name="scr", bufs=2))
        pp = ctx.enter_context(tc.tile_pool(name="ps", bufs=2, space="PSUM"))

        d1s = cpool.tile([128, 128], F16)
        nc.sync.dma_start(d1s[:], d1.ap())
        d2s = cpool.tile([128, 128], F16)
        nc.sync.dma_start(d2s[:], d2.ap())
        acc = cpool.tile([128, 24], F32)
        ones = cpool.tile([128, 1], F16)
        nc.vector.memset(ones[:], 1.0)
        rin_ps = pp.tile([128, 512], F32, name="rin_ps", bufs=1)
        xt2_ps = pp.tile([128, 512], F32, name="xt2_ps", bufs=1)

        for i in range(IMGS):
            # ---- loads ----
            Xp = io.tile([128, 18, 130], F16)
            nc.sync.dma_start(Xp[:, :, 1:129], ypa[i].rearrange("d h w -> h d w"))
            nc.vector.tensor_copy(Xp[:, :, 0], Xp[:, :, 1])
            nc.vector.tensor_copy(Xp[:, :, 129], Xp[:, :, 128])
            xu = io.tile([128, 16, 128], F16)
            nc.sync.dma_start(xu[:], ypa[i, 1:17].rearrange("d h w -> h d w"))
            Tt = io.tile([128, 16, 128], F16)
            nc.sync.dma_start(Tt[:], yta[i].rearrange("d h w -> h d w"))

            # ---- D-axis derivatives (forward-diff route, all 2x aligned) ----
            g = fp.tile([128, 17, 130], F16)
            nc.vector.tensor_sub(g[:], Xp[:, 1:18, :], Xp[:, 0:17, :])
            ci_raw = fp.tile([128, 16, 130], F16)
            nc.vector.tensor_add(ci_raw[:], g[:, 1:17, :], g[:, 0:16, :])
            cii = fp.tile([128, 16, 130], F16)
            nc.vector.tensor_sub(cii[:], g[:, 1:17, :], g[:, 0:16, :])

            # ---- W-axis derivatives (+-2 element offsets stay 4B-aligned) ----
            ck_raw = fp.tile([128, 16, 128], F16)
            nc.vector.tensor_sub(ck_raw[:], Xp[:, 1:17, 2:130], Xp[:, 1:17, 0:128])
            Aw = fp.tile([128, 16, 128], F16)
            nc.vector.tensor_add(Aw[:], Xp[:, 1:17, 2:130], Xp[:, 1:17, 0:128])
            xm2 = sp.tile([128, 16, 128], F16, name="xm2", bufs=1)
            nc.vector.tensor_scalar_mul(xm2[:], xu[:], -2.0)
            ckk = fp.tile([128, 16, 128], F16)
            nc.vector.tensor_add(ckk[:], xm2[:], Aw[:])
            cik_raw = fp.tile([128, 16, 128], F16)
            nc.vector.tensor_sub(cik_raw[:], ci_raw[:, :, 2:130],
                                 ci_raw[:, :, 0:128])

            # ---- H-axis derivatives on the PE ----
            # cj^2 comes straight out of PSUM; cjk = mixed_W(cj) = D1 @ ck_raw
            # (stencils commute), so cj itself is never materialised in SBUF.
            b_t = fp.tile([128, 16, 128], F16)
            for j in range(4):
                ps = pp.tile([128, 4, 128], F32, name="ps_cj", bufs=1)
                nc.tensor.matmul(ps[:], d1s[:], xu[:, 4 * j:4 * j + 4, :],
                                 start=True, stop=True)
                nc.scalar.activation(b_t[:, 4 * j:4 * j + 4, :], ps[:],
                                     Act.Square)
            cjj = fp.tile([128, 16, 128], F16)
            for j in range(4):          # 4 chunks of 4 blocks (512)
                ps2 = pp.tile([128, 4, 128], F32, name="ps_cjj")
                nc.tensor.matmul(ps2[:], d2s[:], xu[:, 4 * j:4 * j + 4, :],
                                 start=True, stop=True)
                nc.scalar.copy(cjj[:, 4 * j:4 * j + 4, :], ps2[:])
            cij = fp.tile([128, 16, 128], F16)
            for j in range(4):
                ps3 = pp.tile([128, 4, 128], F32, name="ps_cij")
                nc.tensor.matmul(ps3[:], d1s[:],
                                 ci_raw[:, 4 * j:4 * j + 4, 1:129],
                                 start=True, stop=True)
                nc.scalar.copy(cij[:, 4 * j:4 * j + 4, :], ps3[:])
            cjk = fp.tile([128, 16, 128], F16)
            for j in range(4):
                ps4 = pp.tile([128, 4, 128], F32, name="ps_cjk", bufs=1)
                nc.tensor.matmul(ps4[:], d1s[:], ck_raw[:, 4 * j:4 * j + 4, :],
                                 start=True, stop=True)
                nc.scalar.copy(cjk[:, 4 * j:4 * j + 4, :], ps4[:])

            # ---- squares: a=(0.5 ci_raw)^2, c=0.25 ck_raw^2 ----
            a_t = fp.tile([128, 16, 128], F16)
            nc.scalar.activation(a_t[:], ci_raw[:, :, 1:129], Act.Square,
                                 scale=0.5)
            c_t = fp.tile([128, 16, 128], F16)
            nc.scalar.activation(c_t[:], ck_raw[:], Act.Square, scale=0.5)

            ab = fp.tile([128, 16, 128], F16)
            nc.vector.tensor_add(ab[:], a_t[:], b_t[:])
            bc = fp.tile([128, 16, 128], F16)
            nc.vector.tensor_add(bc[:], b_t[:], c_t[:])
            ac = fp.tile([128, 16, 128], F16)
            nc.vector.tensor_add(ac[:], a_t[:], c_t[:])
            s2 = fp.tile([128, 16, 128], F16)
            nc.vector.tensor_add(s2[:], ab[:], c_t[:])

            # curv = (1+a+b)ckk + (1+b+c)cii + (1+a+c)cjj - cik_raw*cjk*cij
            ap1 = sp.tile([128, 16, 128], F16, name="ap1", bufs=1)
            nc.vector.tensor_scalar_add(ap1[:], ab[:], 1.0)
            A_t = fp.tile([128, 16, 128], F16)
            nc.vector.tensor_mul(A_t[:], ap1[:], ckk[:])
            B_t = fp.tile([128, 16, 128], F16)
            nc.vector.scalar_tensor_tensor(B_t[:], bc[:], 1.0, cii[:, :, 1:129],
                                           Alu.add, Alu.mult)
            cp1 = sp.tile([128, 16, 128], F16, name="cp1", bufs=1)
            nc.vector.tensor_scalar_add(cp1[:], ac[:], 1.0)
            C_t = fp.tile([128, 16, 128], F16)
            nc.vector.tensor_mul(C_t[:], cp1[:], cjj[:])
            S_t = fp.tile([128, 16, 128], F16)
            nc.vector.tensor_add(S_t[:], A_t[:], B_t[:])
            S2_t = fp.tile([128, 16, 128], F16)
            nc.vector.tensor_add(S2_t[:], S_t[:], C_t[:])
            p1 = fp.tile([128, 16, 128], F16)
            nc.vector.tensor_mul(p1[:], cik_raw[:], cjk[:])
            p2 = fp.tile([128, 16, 128], F16)
            nc.vector.tensor_mul(p2[:], p1[:], cij[:])
            curv = fp.tile([128, 16, 128], F16)
            nc.vector.tensor_sub(curv[:], S2_t[:], p2[:])

            # contrib = curv^2 * length / (1+s)
            rec_in = sp.tile([128, 16, 128], F32, name="rec_in", bufs=1)
            nc.vector.tensor_scalar_add(rec_in[:], s2[:], 1.0)
            r_t = sp.tile([128, 16, 128], F32, name="r_t", bufs=1)
            nc.vector.reciprocal_approx_fast(r_t[:], rec_in[:])
            ln_t = fp.tile([128, 16, 128], F16)
            nc.scalar.activation(ln_t[:], s2[:], Act.Sqrt)
            curv2 = fp.tile([128, 16, 128], F16)
            nc.scalar.activation(curv2[:], curv[:], Act.Square)
            c2l = fp.tile([128, 16, 128], F16)
            nc.vector.tensor_mul(c2l[:], curv2[:], ln_t[:])
            scr = sp.tile([128, 16, 128], F16, name="scr")
            nc.vector.scalar_tensor_tensor(scr[:], c2l[:], 1.0, r_t[:],
                                           Alu.mult, Alu.mult,
                                           accum_out=acc[:, i:i + 1])

            # ---- region sums: products on DVE, reduction on PE (ones-matmul
            # accumulating into a persistent PSUM row across all images) ----
            s1 = fp.tile([128, 16, 128], F16)
            nc.scalar.activation(s1[:], Tt[:], Act.Square, bias=1.0, scale=-1.0)
            rin_p = fp.tile([128, 16, 128], F16)
            nc.vector.tensor_mul(rin_p[:], xu[:], s1[:])
            s2T = fp.tile([128, 16, 128], F16)
            nc.scalar.activation(s2T[:], Tt[:], Act.Square,
                                 accum_out=acc[:, 18 + i:19 + i])
            xt2_p = fp.tile([128, 16, 128], F16)
            nc.vector.tensor_mul(xt2_p[:], xu[:], s2T[:])
            for j in range(4):
                st = (i == 0 and j == 0)
                sto = (i == IMGS - 1 and j == 3)
                nc.tensor.matmul(rin_ps[0:1, :], ones[:],
                                 rin_p[:, 4 * j:4 * j + 4, :],
                                 start=st, stop=sto)
                nc.tensor.matmul(xt2_ps[0:1, :], ones[:],
                                 xt2_p[:, 4 * j:4 * j + 4, :],
                                 start=st, stop=sto)

        nc.vector.tensor_reduce(acc[0:1, 6:7], rin_ps[0:1, :],
                                mybir.AxisListType.X, Alu.add)
        nc.vector.tensor_reduce(acc[0:1, 12:13], xt2_ps[0:1, :],
                                mybir.AxisListType.X, Alu.add)
        nc.sync.dma_start(out.ap(), acc[:])

    nc.compile()
    return nc


def _get_nc():
    if "nc" not in _CACHE:
        _CACHE["nc"] = _build()
    return _CACHE["nc"]


def kernel(y_pred: np.ndarray, y_true: np.ndarray) -> np.ndarray:
    from concourse.bass_utils import run_bass_kernel_spmd

    yp = np.asarray(y_pred).reshape(IMGS, 128, 128, 128)
    yt = np.asarray(y_true).reshape(IMGS, 128, 128, 128)
    d1t, d2t = _stencils()

    in_maps = []
    for c in range(N_CORES):
        idx = np.clip(np.arange(D_SH * c - 1, D_SH * c + D_SH + 1), 0, 127)
        in_maps.append({
            "yp": np.ascontiguousarray(yp[:, idx].astype(np.float16)),
            "yt": np.ascontiguousarray(
                yt[:, D_SH * c:D_SH * (c + 1)].astype(np.float16)),
            "d1": d1t,
            "d2": d2t,
        })

    nc = _get_nc()
    res = run_bass_kernel_spmd(nc, in_maps, core_ids=list(range(N_CORES)))

    elast = rin = t2 = xt2 = 0.0
    for c in range(N_CORES):
        o = res.results[c]["out"].astype(np.float64)
        elast += o[:, 0:6].sum()
        rin += o[0, 6]
        xt2 += o[0, 12]
        t2 += o[:, 18:24].sum()

    total = (MIU * abs(rin) + abs(t2 - xt2)
             + ALPHA * yp.size + BETA * elast)
    return np.array(total, dtype=np.float32)


# revision 24
# speedup vs baseline: 1.2282x; 1.0548x over previous
"""ACELoss3D distributed Trainium2 kernel.

Strategy: pure data-parallel over 8 NeuronCores. The D spatial axis (size 128)
is sharded 8 x 16 with clamp-replicated +-1 halos sliced on the host (clamp
padding reproduces the reference's one-sided boundary formulas exactly, so all
cores run an identical interior-stencil kernel). Inputs are cast to fp16 on the
host (halves DMA traffic; validated ~4e-6 rel err vs the f32 reference).

Device layout per core: 6 images (2x3 batch) x 18 d-slabs; SBUF tiles are
[128 H-partitions, d-blocks x 130] where each 130-wide W block carries
clamp-replicated edge pads so every W stencil is the plain interior stencil.
H-axis derivatives (cj, cjj, cij) are PE matmuls against exact clamped stencil
matrices; D and W derivatives are 2x-mode fp16 DVE ops on +-1-block / +-2-elem
aligned slices. All global sums land in per-partition fp32 accumulators via
fused accum_out; the host reduces the 8 x [128,24] partials.
"""
import sys

sys.path.insert(0, '/opt/trn_rl_repo')

import numpy as np

N_CORES = 8
D_SH = 16          # d-slab owned per core
IMGS = 6           # 2*3 leading dims flattened
ALPHA, BETA, MIU, EPS = 0.001, 1.0, 1.0, 1e-8

_CACHE = {}


def _stencils():
    I = np.eye(128, dtype=np.float64)
    up = I[np.minimum(np.arange(128) + 1, 127)]
    dn = I[np.maximum(np.arange(128) - 1, 0)]
    d1 = (0.5 * (up - dn)).astype(np.float16)
    d2 = (up + dn - 2 * I).astype(np.float16)
    # matmul computes out = lhsT.T @ rhs, so pass D.T as lhsT
    return np.ascontiguousarray(d1.T), np.ascontiguousarray(d2.T)



def _register_recip1p():
    """Custom DVE op: out = approx 1/(in0 + 1) — bit-flip exponent seed +
    one inline Newton-Raphson pass (~0.2% max rel err on [1, 1.76])."""
    import numpy as np
    from concourse import dve_ops as DO
    from concourse.dve_spec import Spec, Src0, C0, C1, C2, Bin, AluOp, lower
    from concourse.dve_uop import DveOpSpec

    NAME = "RECIP1P_ANT"
    for op in DO.OPS:
        if op.name == NAME:
            return op

    def _ref(in0, in1, s0, s1, imm2):
        t = in0.astype(np.float32) + np.float32(imm2)
        nt = (~t.view(np.int32)).view(np.float32)
        y0 = nt * np.float32(s0)
        return (y0 * (np.float32(s1) - t * y0)).astype(np.float32)

    _t = Src0 + C2
    _nt = Bin(AluOp.BITWISE_NOT, _t, _t)
    _y0 = _nt * C0
    spec = Spec(body=_y0 * (C1 - _t * _y0), reference=_ref)
    shas = {}
    for ver in ("v3", "v4"):
        try:
            u = lower(spec, ver=ver)
            shas[ver] = DveOpSpec(name=NAME, opcode=None, uops=u,
                                  rd1_en=False).sha(ver)
        except Exception:
            pass
    op = DO.DveOp(NAME, spec, subdim=False, uops_sha=shas)
    DO.OPS.append(op)
    DO._SUB_OPCODE_FOR_NAME[op.name] = max(DO._SUB_OPCODE_FOR_NAME.values()) + 1
    DO.CUSTOM_DVE_SPECS[op.name] = spec
    return op


def _build():
    import concourse.mybir as mybir
    from concourse import bacc
    from concourse.tile import TileContext
    from contextlib import ExitStack

    F16, F32 = mybir.dt.float16, mybir.dt.float32
    Alu = mybir.AluOpType
    Act = mybir.ActivationFunctionType

    recip_op = _register_recip1p()
    nc = bacc.Bacc("TRN2", target_bir_lowering=False, debug=False,
                   num_devices=N_CORES)
    yp = nc.dram_tensor("yp", [IMGS, D_SH + 2, 128, 128], F16,
                        kind="ExternalInput")
    yt = nc.dram_tensor("yt", [IMGS, D_SH, 128, 128], F16,
                        kind="ExternalInput")
    d1 = nc.dram_tensor("d1", [128, 128], F16, kind="ExternalInput")
    d2 = nc.dram_tensor("d2", [128, 128], F16, kind="ExternalInput")
    out = nc.dram_tensor("out", [128, 24], F32, kind="ExternalOutput")

    ypa, yta = yp.ap(), yt.ap()

    with TileContext(nc) as tc, ExitStack() as ctx:
        cpool = ctx.enter_context(tc.tile_pool(name="const", bufs=1))
        io = ctx.enter_context(tc.tile_pool(name="io", bufs=2))
        fp = ctx.enter_context(tc.tile_pool(name="fields", bufs=1))
        sp = ctx.enter_context(tc.tile_pool(name="scr", bufs=2))
        pp = ctx.enter_context(tc.tile_pool(name="ps", bufs=2, space="PSUM"))

        d1s = cpool.tile([128, 128], F16)
        nc.sync.dma_start(d1s[:], d1.ap())
        d2s = cpool.tile([128, 128], F16)
        nc.sync.dma_start(d2s[:], d2.ap())
        acc = cpool.tile([128, 24], F32)
        ones = cpool.tile([128, 1], F16)
        nc.vector.memset(ones[:], 1.0)
        rin_ps = pp.tile([128, 512], F32, name="rin_ps", bufs=1)
        xt2_ps = pp.tile([128, 512], F32, name="xt2_ps", bufs=1)
        el_ps = pp.tile([128, 512], F32, name="el_ps", bufs=1)

        for i in range(IMGS):
            # ---- loads ----
            Xp = io.tile([128, 18, 130], F16)
            nc.sync.dma_start(Xp[:, :, 1:129], ypa[i].rearrange("d h w -> h d w"))
            nc.vector.tensor_copy(Xp[:, :, 0], Xp[:, :, 1])
            nc.vector.tensor_copy(Xp[:, :, 129], Xp[:, :, 128])
            xu = io.tile([128, 16, 128], F16)
            nc.sync.dma_start(xu[:], ypa[i, 1:17].rearrange("d h w -> h d w"))
            Tt = io.tile([128, 16, 128], F16)
            nc.sync.dma_start(Tt[:], yta[i].rearrange("d h w -> h d w"))

            # ---- D-axis derivatives (forward-diff route, all 2x aligned) ----
            g = fp.tile([128, 17, 130], F16)
            nc.vector.tensor_sub(g[:], Xp[:, 1:18, :], Xp[:, 0:17, :])
            ci_raw = fp.tile([128, 16, 130], F16)
            nc.vector.tensor_add(ci_raw[:], g[:, 1:17, :], g[:, 0:16, :])
            cii = fp.tile([128, 16, 130], F16)
            nc.vector.tensor_sub(cii[:], g[:, 1:17, :], g[:, 0:16, :])

            # ---- W-axis derivatives (+-2 element offsets stay 4B-aligned) ----
            ck_raw = fp.tile([128, 16, 128], F16)
            nc.vector.tensor_sub(ck_raw[:], Xp[:, 1:17, 2:130], Xp[:, 1:17, 0:128])
            Aw = fp.tile([128, 16, 128], F16)
            nc.vector.tensor_add(Aw[:], Xp[:, 1:17, 2:130], Xp[:, 1:17, 0:128])
            xm2 = sp.tile([128, 16, 128], F16, name="xm2", bufs=1)
            nc.vector.tensor_scalar_mul(xm2[:], xu[:], -2.0)
            ckk = fp.tile([128, 16, 128], F16)
            nc.vector.tensor_add(ckk[:], xm2[:], Aw[:])
            cik_raw = fp.tile([128, 16, 128], F16)
            nc.vector.tensor_sub(cik_raw[:], ci_raw[:, :, 2:130],
                                 ci_raw[:, :, 0:128])

            # ---- H-axis derivatives on the PE ----
            # cj^2 comes straight out of PSUM; cjk = mixed_W(cj) = D1 @ ck_raw
            # (stencils commute), so cj itself is never materialised in SBUF.
            b_t = fp.tile([128, 16, 128], F16)
            for j in range(4):
                ps = pp.tile([128, 4, 128], F32, name="ps_cj", bufs=1)
                nc.tensor.matmul(ps[:], d1s[:], xu[:, 4 * j:4 * j + 4, :],
                                 start=True, stop=True)
                nc.scalar.activation(b_t[:, 4 * j:4 * j + 4, :], ps[:],
                                     Act.Square)
            cjj = fp.tile([128, 16, 128], F16)
            for j in range(4):          # 4 chunks of 4 blocks (512)
                ps2 = pp.tile([128, 4, 128], F32, name="ps_cjj", bufs=1)
                nc.tensor.matmul(ps2[:], d2s[:], xu[:, 4 * j:4 * j + 4, :],
                                 start=True, stop=True)
                nc.scalar.copy(cjj[:, 4 * j:4 * j + 4, :], ps2[:])
            cij = fp.tile([128, 16, 128], F16)
            for j in range(4):
                ps3 = pp.tile([128, 4, 128], F32, name="ps_cij")
                nc.tensor.matmul(ps3[:], d1s[:],
                                 ci_raw[:, 4 * j:4 * j + 4, 1:129],
                                 start=True, stop=True)
                nc.scalar.copy(cij[:, 4 * j:4 * j + 4, :], ps3[:])
            cjk = fp.tile([128, 16, 128], F16)
            for j in range(4):
                ps4 = pp.tile([128, 4, 128], F32, name="ps_cjk", bufs=1)
                nc.tensor.matmul(ps4[:], d1s[:], ck_raw[:, 4 * j:4 * j + 4, :],
                                 start=True, stop=True)
                nc.scalar.copy(cjk[:, 4 * j:4 * j + 4, :], ps4[:])

            # ---- squares: a=(0.5 ci_raw)^2, c=0.25 ck_raw^2 ----
            a_t = fp.tile([128, 16, 128], F16)
            nc.scalar.activation(a_t[:], ci_raw[:, :, 1:129], Act.Square,
                                 scale=0.5)
            c_t = fp.tile([128, 16, 128], F16)
            nc.scalar.activation(c_t[:], ck_raw[:], Act.Square, scale=0.5)

            ab = fp.tile([128, 16, 128], F16)
            nc.vector.tensor_add(ab[:], a_t[:], b_t[:])
            bc = fp.tile([128, 16, 128], F16)
            nc.vector.tensor_add(bc[:], b_t[:], c_t[:])
            ac = fp.tile([128, 16, 128], F16)
            nc.vector.tensor_add(ac[:], a_t[:], c_t[:])
            s2 = fp.tile([128, 16, 128], F16)
            nc.vector.tensor_add(s2[:], ab[:], c_t[:])

            # curv = (1+a+b)ckk + (1+b+c)cii + (1+a+c)cjj - cik_raw*cjk*cij
            ap1 = sp.tile([128, 16, 128], F16, name="ap1", bufs=1)
            nc.vector.tensor_scalar_add(ap1[:], ab[:], 1.0)
            A_t = fp.tile([128, 16, 128], F16)
            nc.vector.tensor_mul(A_t[:], ap1[:], ckk[:])
            B_t = fp.tile([128, 16, 128], F16)
            nc.vector.scalar_tensor_tensor(B_t[:], bc[:], 1.0, cii[:, :, 1:129],
                                           Alu.add, Alu.mult)
            cp1 = sp.tile([128, 16, 128], F16, name="cp1", bufs=1)
            nc.vector.tensor_scalar_add(cp1[:], ac[:], 1.0)
            C_t = fp.tile([128, 16, 128], F16)
            nc.vector.tensor_mul(C_t[:], cp1[:], cjj[:])
            S_t = fp.tile([128, 16, 128], F16)
            nc.vector.tensor_add(S_t[:], A_t[:], B_t[:])
            S2_t = fp.tile([128, 16, 128], F16)
            nc.vector.tensor_add(S2_t[:], S_t[:], C_t[:])
            p1 = fp.tile([128, 16, 128], F16)
            nc.vector.tensor_mul(p1[:], cik_raw[:], cjk[:])
            p2 = fp.tile([128, 16, 128], F16)
            nc.vector.tensor_mul(p2[:], p1[:], cij[:])
            curv = fp.tile([128, 16, 128], F16)
            nc.vector.tensor_sub(curv[:], S2_t[:], p2[:])

            # contrib = curv^2 * length / (1+s); 1/(1+s) via one custom DVE op
            r16 = sp.tile([128, 16, 128], F16, name="r16", bufs=1)
            nc.vector._custom_dve(recip_op, out=r16[:], in0=s2[:],
                                  s0=-0.23549792, s1=2.0017324, imm2=1.0)
            ln_t = fp.tile([128, 16, 128], F16)
            nc.scalar.activation(ln_t[:], s2[:], Act.Sqrt)
            curv2 = fp.tile([128, 16, 128], F16)
            nc.scalar.activation(curv2[:], curv[:], Act.Square)
            c2l = fp.tile([128, 16, 128], F16)
            nc.vector.tensor_mul(c2l[:], curv2[:], ln_t[:])
            contrib = fp.tile([128, 16, 128], F16)
            nc.vector.tensor_mul(contrib[:], c2l[:], r16[:])
            for j in range(4):
                nc.tensor.matmul(el_ps[0:1, :], ones[:],
                                 contrib[:, 4 * j:4 * j + 4, :],
                                 start=(i == 0 and j == 0),
                                 stop=(i == IMGS - 1 and j == 3))

            # ---- region sums: products on DVE, reduction on PE (ones-matmul
            # accumulating into a persistent PSUM row across all images) ----
            s1 = fp.tile([128, 16, 128], F16)
            nc.scalar.activation(s1[:], Tt[:], Act.Square, bias=1.0, scale=-1.0)
            rin_p = fp.tile([128, 16, 128], F16)
            nc.vector.tensor_mul(rin_p[:], xu[:], s1[:])
            s2T = fp.tile([128, 16, 128], F16)
            nc.scalar.activation(s2T[:], Tt[:], Act.Square,
                                 accum_out=acc[:, 18 + i:19 + i])
            xt2_p = fp.tile([128, 16, 128], F16)
            nc.vector.tensor_mul(xt2_p[:], xu[:], s2T[:])
            for j in range(4):
                st = (i == 0 and j == 0)
                sto = (i == IMGS - 1 and j == 3)
                nc.tensor.matmul(rin_ps[0:1, :], ones[:],
                                 rin_p[:, 4 * j:4 * j + 4, :],
                                 start=st, stop=sto)
                nc.tensor.matmul(xt2_ps[0:1, :], ones[:],
                                 xt2_p[:, 4 * j:4 * j + 4, :],
                                 start=st, stop=sto)

        nc.vector.tensor_reduce(acc[0:1, 0:1], el_ps[0:1, :],
                                mybir.AxisListType.X, Alu.add)
        nc.vector.tensor_reduce(acc[0:1, 6:7], rin_ps[0:1, :],
                                mybir.AxisListType.X, Alu.add)
        nc.vector.tensor_reduce(acc[0:1, 12:13], xt2_ps[0:1, :],
                                mybir.AxisListType.X, Alu.add)
        nc.sync.dma_start(out.ap(), acc[:])

    nc.compile()
    return nc


def _get_nc():
    if "nc" not in _CACHE:
        _CACHE["nc"] = _build()
    return _CACHE["nc"]


def kernel(y_pred: np.ndarray, y_true: np.ndarray) -> np.ndarray:
    from concourse.bass_utils import run_bass_kernel_spmd

    yp = np.asarray(y_pred).reshape(IMGS, 128, 128, 128)
    yt = np.asarray(y_true).reshape(IMGS, 128, 128, 128)
    d1t, d2t = _stencils()

    in_maps = []
    for c in range(N_CORES):
        idx = np.clip(np.arange(D_SH * c - 1, D_SH * c + D_SH + 1), 0, 127)
        in_maps.append({
            "yp": np.ascontiguousarray(yp[:, idx].astype(np.float16)),
            "yt": np.ascontiguousarray(
                yt[:, D_SH * c:D_SH * (c + 1)].astype(np.float16)),
            "d1": d1t,
            "d2": d2t,
        })

    nc = _get_nc()
    res = run_bass_kernel_spmd(nc, in_maps, core_ids=list(range(N_CORES)))

    elast = rin = t2 = xt2 = 0.0
    for c in range(N_CORES):
        o = res.results[c]["out"].astype(np.float64)
        elast += o[0, 0]
        rin += o[0, 6]
        xt2 += o[0, 12]
        t2 += o[:, 18:24].sum()

    total = (MIU * abs(rin) + abs(t2 - xt2)
             + ALPHA * yp.size + BETA * elast)
    return np.array(total, dtype=np.float32)
